# revision 1
# baseline (speedup 1.0000x reference)
"""Trainium2 Bass kernel v2: GQA attention (nn_Attention), TP8 over heads.

Single fused pipeline per core (core c owns kv head c, q heads 4c..4c+3):

  W1  K/V projection for batch 0 (pure PE), rope K at eviction, V
      transposed to natural layout via identity matmuls.
  Wb  per batch: for each 512-token query block a:
        JIT Q projection (bf16 x chunks re-read from DRAM), rope on DVE,
        attention (scores f32r -> exp bf16 on ACT -> AV bf16), softmax
        denominator summed on DVE/Pool ping-pong + one PE ones-matmul,
        o_proj of the PREVIOUS block woven into the ACT-bound attention
        slots as PE filler (plus Q proj of next block / KV proj of the
        next batch) via a FIFO filler queue at ~2-matmul granularity.
  Tail  drain remaining o_proj groups.

Precision: x/W bf16 (PE rate identical to f32r), Q/K evictions + rope +
scores in f32r so the softmax logits only carry the bf16 input
quantization; P/V/O/y in bf16.  All matmul accumulation in f32 PSUM.
Host sums the 8 bf16 partial outputs in f64.

No QKV DRAM roundtrip: K/V live in SBUF per batch; Q is projected on
demand (x is re-read, DMA has big slack).  Weights/constants are loaded
once and stay resident; with everything bf16 the whole working set fits
in SBUF, which lets consecutive timing-loop iterations overlap.
"""

import math
from contextlib import ExitStack, nullcontext

import numpy as np

import concourse.bass as bass
import concourse.tile as tile
from concourse import bacc
from concourse import mybir
from concourse.bass import ts, ds

# Problem constants (hardcoded; kernel.py must be self-contained).
HIDDEN = 4096
N_HEADS = 32
N_KV_HEADS = 8
D = 128                      # head dim
B = 2
S = 2048
N_CORES = 8
QH = N_HEADS // N_CORES      # q heads per core = 4
ROPE_THETA = 10000.0
SCALE = 1.0 / math.sqrt(D)

F32 = mybir.dt.float32
F32R = mybir.dt.float32r
BF16 = mybir.dt.bfloat16
EXP = mybir.ActivationFunctionType.Exp

AB = 512                     # token block (query block, projection block)
KC = 8                       # kt tiles per x chunk
KT_N = HIDDEN // 128         # 32 contraction tiles
NCH = KT_N // KC             # 4 chunks per block
SK = S // 128                # 16 key tiles per batch
NAB = S // AB                # 4 query blocks per batch
NBLK = (B * S) // AB         # 8 token blocks total
TOK = B * S


class Filler:
    """FIFO queue of emission generators, drained head-first."""

    def __init__(self):
        self.q = []
        self.units = 0

    def push(self, gen, n_units):
        self.q.append(gen)
        self.units += n_units

    def pull(self, n):
        """Advance head generator(s) by n yields."""
        while n > 0 and self.q:
            try:
                next(self.q[0])
                self.units -= 1
                n -= 1
            except StopIteration:
                self.q.pop(0)
        return n

    def drain(self):
        while self.q:
            try:
                next(self.q[0])
                self.units -= 1
            except StopIteration:
                self.q.pop(0)


MARKS = []


def build_nc(timing_loop=None, tune=None, marks=False):
    MARKS.clear()

    tn = dict(xb=3, pT=4, qT=8, oTb=10, y=3, rb=2, dacc=2, rot=2,
              qe=2, ke=2, vT=2, kT=2, vsb=2,
              qp=2, pss=2, pso=2, psy=2, cap=8, gpb=False, den_pool=False,
              skip_den=False, pipe=False, rbbc=False)
    if tune:
        tn.update(tune)

    hid, s, b, qh = HIDDEN, S, B, QH
    qdim = qh * 128

    nc = bacc.Bacc("TRN2", target_bir_lowering=False, debug=False)

    def mark(label):
        if marks:
            MARKS.append((int(nc.next_id()), label))

    big = "Internal" if timing_loop else "ExternalInput"
    # x chunks: xt[blk, c] is a contiguous [128, KC, AB] bf16 block,
    # xt[blk,c,p,j,t] = X[blk*AB + t, (c*KC+j)*128 + p]
    xt = nc.dram_tensor("xt", [NBLK, NCH, 128, KC, AB], BF16, kind=big)
    wqt = nc.dram_tensor("wqt", [128, KT_N, qdim], BF16, kind=big)
    wkt = nc.dram_tensor("wkt", [128, KT_N, 128], BF16, kind=big)
    wvt = nc.dram_tensor("wvt", [128, KT_N, 128], BF16, kind=big)
    wot = nc.dram_tensor("wot", [qh, 128, hid], BF16, kind=big)
    cos_d = nc.dram_tensor("cos_t", [128, s], BF16, kind="ExternalInput")
    sin_d = nc.dram_tensor("sin_t", [128, s], BF16, kind="ExternalInput")  # sign-baked
    ident_d = nc.dram_tensor("ident", [128, 128], BF16, kind="ExternalInput")
    ones_d = nc.dram_tensor("ones", [128, 128], F32R, kind="ExternalInput")
    if timing_loop:
        yt = nc.dram_tensor("yt", [hid, TOK], BF16, kind="Internal")
        yt_small = nc.dram_tensor("yt_small", [128, 128], BF16, kind="ExternalOutput")
    else:
        yt = nc.dram_tensor("yt", [hid, TOK], BF16, kind="ExternalOutput")

    with tile.TileContext(nc) as tc, ExitStack() as top:
        # ---------------- persistent weights + constants -------------------
        wpool = top.enter_context(tc.tile_pool(name="wts", bufs=1))
        wq_sb = wpool.tile([128, KT_N, qdim], BF16, name="wq_sb")
        wk_sb = wpool.tile([128, KT_N, 128], BF16, name="wk_sb")
        wv_sb = wpool.tile([128, KT_N, 128], BF16, name="wv_sb")
        wo_sb = [wpool.tile([128, hid], BF16, name=f"wo_sb{dv}") for dv in range(qh)]
        cos_sb = wpool.tile([128, s], BF16, name="cos_sb")
        sin_sb = wpool.tile([128, s], BF16, name="sin_sb")
        ident = wpool.tile([128, 128], BF16, name="ident")
        ones_sb = wpool.tile([128, 128], F32R, name="ones_sb")
        ones_col = ones_sb[:, 0:1]
        ones_row = ones_sb[0:1, :]

        if timing_loop:
            # Zero-fill weight SBUF directly + xt DRAM once.
            nc.gpsimd.memset(wq_sb[:], 0.0)
            nc.gpsimd.memset(wk_sb[:], 0.0)
            nc.gpsimd.memset(wv_sb[:], 0.0)
            for dv in range(qh):
                nc.gpsimd.memset(wo_sb[dv][:], 0.0)
            with tc.tile_pool(name="zero", bufs=1) as zp:
                zt = zp.tile([128, KC, AB], BF16, name="zt")
                nc.vector.memset(zt[:], 0.0)
                for blk in range(NBLK):
                    for c in range(NCH):
                        nc.sync.dma_start(xt[blk, c], zt[:])
        else:
            # K/V weights first so the first projection can start ASAP;
            # Wo last (first needed ~200us in).
            for c0 in range(0, KT_N, 8):
                nc.sync.dma_start(wk_sb[:, c0:c0 + 8, :], wkt[:, c0:c0 + 8, :])
                nc.sync.dma_start(wv_sb[:, c0:c0 + 8, :], wvt[:, c0:c0 + 8, :])
            for kt in range(KT_N):
                nc.sync.dma_start(wq_sb[:, kt, :], wqt[:, kt, :])
            for dv in range(qh):
                nc.sync.dma_start(wo_sb[dv][:], wot[dv])
        nc.sync.dma_start(cos_sb[:], cos_d.ap())
        nc.sync.dma_start(sin_sb[:], sin_d.ap())
        nc.sync.dma_start(ident[:], ident_d.ap())
        nc.sync.dma_start(ones_sb[:], ones_d.ap())

        pipe = tn["pipe"] and timing_loop
        xpool = top.enter_context(tc.tile_pool(name="x", bufs=tn["xb"]))
        kvpool = top.enter_context(tc.tile_pool(name="kv", bufs=1))
        qpool = top.enter_context(tc.tile_pool(name="q", bufs=1))
        ppool = top.enter_context(tc.tile_pool(name="pT", bufs=tn["pT"]))
        smpool = top.enter_context(tc.tile_pool(name="sm", bufs=1))
        opool = top.enter_context(tc.tile_pool(name="oT", bufs=tn["oTb"]))
        ypool = top.enter_context(tc.tile_pool(name="y", bufs=tn["y"]))
        ps = top.enter_context(tc.tile_pool(name="ps", bufs=1, space="PSUM"))
        loop_cm = tc.For_i(0, timing_loop, 1) if timing_loop else nullcontext()
        if True:

            # per-batch persistent K/V (ring of 2 so next batch can prefetch)
            kT = {}
            v_sb = {}

            def rope_block(src, dst, pos0):
                """dst[:, :AB] = src*cos + rot_half(src)*sin for a 512-token
                block starting at position pos0 (within the batch)."""
                rot = qpool.tile([128, AB], F32R, tag="rot", bufs=tn["rot"],
                                 name="rot")
                nc.sync.dma_start(rot[0:64, :], src[64:128, :])
                nc.sync.dma_start(rot[64:128, :], src[0:64, :])
                nc.vector.tensor_mul(dst, src, cos_sb[:, ds(pos0, AB)])
                nc.vector.tensor_mul(rot[:], rot[:], sin_sb[:, ds(pos0, AB)])
                nc.vector.tensor_add(dst, dst, rot[:])

            def kvproj_gen(bb, kT_out=None, v_out=None):
                """K/V projection + K rope + V transpose for batch bb.
                K^T/V^T computed per 512-block; K roped into kT[bb],
                V transposed into v_sb[bb] (natural [key, dv])."""
                kT[bb] = kT_out if kT_out is not None else kvpool.tile(
                    [128, s], F32R, tag="kT", bufs=tn["kT"], name="kT")
                v_sb[bb] = v_out if v_out is not None else kvpool.tile(
                    [128, s], BF16, tag="vsb", bufs=tn["vsb"], name="v_sb")
                for ablk in range(NAB):
                    mark(f"kvproj b{bb} blk{ablk}")
                    blk = bb * NAB + ablk
                    psK = ps.tile([128, AB], F32, tag="qp", bufs=tn["qp"],
                                  name="psK")
                    psV = ps.tile([128, AB], F32, tag="qp", bufs=tn["qp"],
                                  name="psV")
                    for c in range(NCH):
                        x_sb = xpool.tile([128, KC, AB], BF16, tag="x",
                                          name="x_sb")
                        nc.sync.dma_start(x_sb[:], xt[blk, c])
                        for j in range(KC):
                            kt = c * KC + j
                            nc.tensor.matmul(
                                psK[:], wk_sb[:, kt, :], x_sb[:, j, :],
                                start=(kt == 0), stop=(kt == KT_N - 1))
                            nc.tensor.matmul(
                                psV[:], wv_sb[:, kt, :], x_sb[:, j, :],
                                start=(kt == 0), stop=(kt == KT_N - 1))
                            if j % 2 == 1:
                                yield
                    # K: evict f32r, rope into resident kT
                    ke = qpool.tile([128, AB], F32R, tag="ke", bufs=tn["ke"],
                                    name="ke")
                    nc.scalar.copy(ke[:], psK[:])
                    rope_block(ke[:], kT[bb][:, ts(ablk, AB)], ablk * AB)
                    yield
                    # V: evict bf16, transpose 4 key-tiles to natural layout
                    vT = qpool.tile([128, AB], BF16, tag="vT", bufs=tn["vT"],
                                    name="vT")
                    nc.scalar.copy(vT[:], psV[:])
                    for k4 in range(4):
                        psv = ps.tile([128, 128], F32, tag="pss", bufs=tn["pss"],
                                      name="psv")
                        nc.tensor.matmul(psv[:], vT[:, ts(k4, 128)], ident[:],
                                         start=True, stop=True)
                        k2 = ablk * 4 + k4
                        nc.scalar.copy(v_sb[bb][:, ts(k2, 128)], psv[:])
                        yield

            def qproj_gen(bb, a, outs=None):
                """JIT Q projection for query block (bb, a): all 4 heads,
                f32r eviction + rope -> qT tiles [128, AB]."""
                blk = bb * NAB + a
                out = []
                for h2 in range(0, qh, 2):
                    mark(f"qproj b{bb} a{a} h{h2}")
                    psA = ps.tile([128, AB], F32, tag="qp", bufs=tn["qp"],
                                  name="psA")
                    psB = ps.tile([128, AB], F32, tag="qp", bufs=tn["qp"],
                                  name="psB")
                    for c in range(NCH):
                        x_sb = xpool.tile([128, KC, AB], BF16, tag="x",
                                          name="x_sb")
                        nc.sync.dma_start(x_sb[:], xt[blk, c])
                        for j in range(KC):
                            kt = c * KC + j
                            nc.tensor.matmul(
                                psA[:], wq_sb[:, kt, ts(h2, 128)],
                                x_sb[:, j, :],
                                start=(kt == 0), stop=(kt == KT_N - 1))
                            nc.tensor.matmul(
                                psB[:], wq_sb[:, kt, ts(h2 + 1, 128)],
                                x_sb[:, j, :],
                                start=(kt == 0), stop=(kt == KT_N - 1))
                            if j % 2 == 1:
                                yield
                    for h, psq in ((h2, psA), (h2 + 1, psB)):
                        qe = qpool.tile([128, AB], F32R, tag="qe", bufs=tn["qe"],
                                        name="qe")
                        nc.scalar.copy(qe[:], psq[:])
                        if outs is not None:
                            qT = outs[h]
                        else:
                            qT = qpool.tile([128, AB], F32R, tag="qT",
                                            bufs=tn["qT"], name="qT")
                        rope_block(qe[:], qT[:], a * AB)
                        out.append(qT)
                        yield
                qT_blk[(bb, a)] = out

            def oproj_gen(bb, a):
                """o_proj for query block (bb, a): y[:, block] partial from the
                4 per-head oT blocks."""
                oTs = oT_blk.pop((bb, a))
                for ht2 in range(0, hid // 128, 2):
                    mark(f"oproj b{bb} a{a} ht{ht2}")
                    y_sb = ypool.tile([128, 2, AB], BF16, tag="y", name="y_sb")
                    for u in range(2):
                        ht = ht2 + u
                        ps_y = ps.tile([128, AB], F32, tag="psy", bufs=tn["psy"],
                                       name="ps_y")
                        for dv in range(qh):
                            nc.tensor.matmul(
                                ps_y[:], wo_sb[dv][:, ts(ht, 128)], oTs[dv][:],
                                start=(dv == 0), stop=(dv == qh - 1))
                        # rotate evictions ACT/DVE
                        if ht % 2 == 0:
                            nc.scalar.copy(y_sb[:, u, :], ps_y[:])
                        else:
                            nc.vector.tensor_copy(y_sb[:, u, :], ps_y[:])
                        yield
                    nc.sync.dma_start(
                        yt.ap()[ts(ht2 // 2, 256),
                                ds(bb * s + a * AB, AB)].rearrange(
                                    "(u p) t -> p u t", u=2),
                        y_sb[:])

            qT_blk = {}
            oT_blk = {}
            fill = Filler()

            def attn_block(bb, a):
                """Attention for query block (bb, a), all 4 heads; weaves
                filler MMs into the ACT-bound slots."""
                nslots = qh * (SK + 3)
                ratio = min(tn["cap"], fill.units / nslots)
                acc = 0.0
                oTs = []
                for h in range(qh):
                    mark(f"attn b{bb} a{a} h{h}")
                    qT = qT_blk[(bb, a)][h]
                    ps_o = ps.tile([128, AB], F32, tag="pso", bufs=tn["pso"],
                                   name="ps_o")
                    daccs = [(nc.vector,
                              smpool.tile([128, AB], F32R, tag="daccv",
                                          bufs=tn["dacc"], name="dacc_v"))]
                    if tn["den_pool"]:
                        daccs.append((nc.gpsimd,
                                      smpool.tile([128, AB], F32R, tag="daccp",
                                                  bufs=tn["dacc"],
                                                  name="dacc_p")))
                    for k2 in range(SK):
                        ps_s = ps.tile([128, AB], F32, tag="pss", bufs=tn["pss"],
                                       name="ps_s")
                        nc.tensor.matmul(ps_s[:], kT[bb][:, ts(k2, 128)],
                                         qT[:], start=True, stop=True)
                        pT = ppool.tile([128, AB], BF16, tag="pT", name="pT")
                        nc.scalar.activation(pT[:], ps_s[:], EXP, scale=SCALE)
                        nc.tensor.matmul(ps_o[:], v_sb[bb][:, ts(k2, 128)],
                                         pT[:],
                                         start=(k2 == 0), stop=(k2 == SK - 1))
                        if not tn["skip_den"]:
                            eng, dacc = daccs[k2 % len(daccs)]
                            if k2 < len(daccs):
                                eng.tensor_copy(dacc[:], pT[:])
                            else:
                                eng.tensor_add(dacc[:], dacc[:], pT[:])
                        # weave filler into the ACT-bound slot
                        acc += ratio
                        n = int(acc)
                        if n:
                            acc -= n
                            fill.pull(n)
                    # denominator: partition-reduce both accumulators on PE
                    mark(f"dentail b{bb} a{a} h{h}")
                    def slot():
                        nonlocal acc
                        acc += ratio
                        n = int(acc)
                        if n:
                            acc -= n
                            fill.pull(n)
                    if tn["skip_den"]:
                        oT = opool.tile([128, AB], BF16, tag="oT", name="oT")
                        nc.vector.tensor_copy(oT[:], ps_o[:])
                        oTs.append(oT)
                        for _ in range(4):
                            slot()
                        continue
                    ps_den = ps.tile([1, AB], F32, tag="qp", bufs=tn["qp"],
                                     name="ps_den")
                    for i, (_, dacc) in enumerate(daccs):
                        nc.tensor.matmul(ps_den[:], ones_col, dacc[:],
                                         start=(i == 0),
                                         stop=(i == len(daccs) - 1))
                    slot()
                    rcp = smpool.tile([1, AB], F32R, tag="rcp", bufs=1,
                                      name="rcp")
                    with nc.allow_low_precision(reason="f32r softmax den"):
                        nc.vector.reciprocal(rcp[:], ps_den[:])
                    slot()
                    rb = smpool.tile([128, AB], F32R, tag="rb",
                                     bufs=tn["rb"], name="rb")
                    if tn["rbbc"]:
                        nc.sync.dma_start(rb[:], rcp[:].partition_broadcast(128))
                    elif tn["gpb"]:
                        nc.gpsimd.partition_broadcast(rb[:], rcp[:])
                    if not tn["rbbc"] and not tn["gpb"]:
                        ps_rb = ps.tile([128, AB], F32, tag="psy",
                                        bufs=tn["psy"], name="ps_rb")
                        nc.tensor.matmul(ps_rb[:], ones_row, rcp[:],
                                         start=True, stop=True)
                        nc.scalar.copy(rb[:], ps_rb[:])
                    slot()
                    oT = opool.tile([128, AB], BF16, tag="oT", name="oT")
                    nc.vector.tensor_mul(oT[:], ps_o[:], rb[:])
                    oTs.append(oT)
                oT_blk[(bb, a)] = oTs

            # ----------------------- schedule --------------------------------
            kv_done = set()

            def kv_wrap(bb, kT_out=None, v_out=None):
                yield from kvproj_gen(bb, kT_out, v_out)
                kv_done.add(bb)

            q_units = 2 * (NCH * KC // 2 + 2)          # yields per qproj gen
            kv_units = NAB * (NCH * KC // 2 + 5)       # yields per kvproj gen
            o_units = hid // 128                       # yields per oproj gen

            def prologue():
                # K/V batch 0 + first Q block
                for _ in kv_wrap(0):
                    pass
                for _ in qproj_gen(0, 0):
                    pass

            def body():
                for bb in range(b):
                    for a in range(NAB):
                        if a + 1 < NAB:
                            fill.push(qproj_gen(bb, a + 1), q_units)
                        elif bb + 1 < b:
                            fill.push(kv_wrap(bb + 1), kv_units)
                            fill.push(qproj_gen(bb + 1, 0), q_units)
                        elif pipe:
                            # rotate next iteration's prologue into this tail,
                            # writing into the pre-allocated batch-0 tiles
                            fill.push(kv_wrap(0, kT[0], v_sb[0]), kv_units)
                            fill.push(qproj_gen(0, 0, qT_blk[(0, 0)]), q_units)
                        # emission-order guard: q (and kv) for this block must
                        # be fully emitted before attention reads them
                        while (bb, a) not in qT_blk or bb not in kv_done:
                            left = fill.pull(1)
                            assert left == 0, "filler queue empty, deps missing"
                        attn_block(bb, a)
                        fill.push(oproj_gen(bb, a), o_units)
                fill.drain()

            if pipe:
                # Software-pipelined steady state: each iteration's batch-0
                # K/V/Q-block-0 work runs in the previous iteration's tail.
                # Pre-allocate those tiles; the first iteration reads them
                # unwritten (timing build only; the cold first iteration
                # cancels in the R2-R1 difference).
                with loop_cm:
                    kT[0] = kvpool.tile([128, s], F32R, tag="kT",
                                        bufs=tn["kT"], name="kT")
                    v_sb[0] = kvpool.tile([128, s], BF16, tag="vsb",
                                          bufs=tn["vsb"], name="v_sb")
                    qT_blk[(0, 0)] = [
                        qpool.tile([128, AB], F32R, tag="qT", bufs=tn["qT"],
                                   name="qT")
                        for _ in range(qh)]
                    nc.vector.memset(kT[0][:].bitcast(F32), 0.0)
                    nc.vector.memset(v_sb[0][:], 0.0)
                    for t_ in qT_blk[(0, 0)]:
                        nc.vector.memset(t_[:].bitcast(F32), 0.0)
                    kv_done.add(0)
                    body()
            else:
                with loop_cm:
                    prologue()
                    body()

        if timing_loop:
            with tc.tile_pool(name="smallout", bufs=1) as sp:
                t = sp.tile([128, 128], BF16, name="t_small")
                nc.sync.dma_start(t[:], yt.ap()[0:128, 0:128])
                nc.sync.dma_start(yt_small.ap()[:, :], t[:])

    nc.compile()
    return nc


# ----------------------------------------------------------------------------
# Host side
# ----------------------------------------------------------------------------

def _rope_tables(position_ids, s):
    """cos^T/sin^T tables [128, s] in d-on-partition layout; sin sign-baked."""
    pos = np.asarray(position_ids).reshape(-1).astype(np.float64)
    assert pos.shape[0] == s
    inv = 1.0 / (ROPE_THETA ** (np.arange(0, D, 2, dtype=np.float64) / D))  # [64]
    f = inv[:, None] * pos[None, :]                      # [64, s]
    ff = np.concatenate([f, f], axis=0)                  # [128, s]
    cos_t = np.cos(ff).astype(np.float32)
    sin_t = np.sin(ff).astype(np.float32)
    sin_t[:64] *= -1.0                                   # rot[0:64] = -q[64:128]
    return np.ascontiguousarray(cos_t), np.ascontiguousarray(sin_t)


def _prep_in_maps(hidden_states, position_ids, Wq, Wk, Wv, Wo):
    import ml_dtypes
    bf16 = ml_dtypes.bfloat16
    s, qh, hid = S, QH, HIDDEN
    qdim = qh * 128

    X = np.asarray(hidden_states, dtype=np.float32).reshape(TOK, hid)
    # xt[blk, c, p, j, t] = X[blk*AB + t, (c*KC + j)*128 + p]
    xt = np.ascontiguousarray(
        X.reshape(NBLK, AB, NCH, KC, 128).transpose(0, 2, 4, 3, 1)
    ).astype(bf16)
    cos_t, sin_t = _rope_tables(position_ids, s)

    Wq = np.asarray(Wq, dtype=np.float32)
    Wk = np.asarray(Wk, dtype=np.float32)
    Wv = np.asarray(Wv, dtype=np.float32)
    Wo = np.asarray(Wo, dtype=np.float32)

    maps = []
    for c in range(N_CORES):
        wq = Wq[c * qdim:(c + 1) * qdim].T                 # [hid, qdim]
        wqt = np.ascontiguousarray(
            wq.reshape(KT_N, 128, qdim).transpose(1, 0, 2)).astype(bf16)
        wk = Wk[c * 128:(c + 1) * 128].T
        wkt = np.ascontiguousarray(
            wk.reshape(KT_N, 128, 128).transpose(1, 0, 2)).astype(bf16)
        wv = Wv[c * 128:(c + 1) * 128].T
        wvt = np.ascontiguousarray(
            wv.reshape(KT_N, 128, 128).transpose(1, 0, 2)).astype(bf16)
        wo = np.ascontiguousarray(Wo[:, c * qdim:(c + 1) * qdim].T)  # [qdim, hid]
        wot = wo.reshape(qh, 128, hid).astype(bf16)
        maps.append({
            "xt": xt, "wqt": wqt, "wkt": wkt, "wvt": wvt, "wot": wot,
            "cos_t": cos_t.astype(bf16), "sin_t": sin_t.astype(bf16),
            "ident": np.eye(128, dtype=np.float32).astype(bf16),
            "ones": np.ones((128, 128), dtype=np.float32),
        })
    return maps


_NC_CACHE = {}


def _get_nc():
    if "nc" not in _NC_CACHE:
        _NC_CACHE["nc"] = build_nc()
    return _NC_CACHE["nc"]


def run(inputs, trace=False, **kw):
    """Run the SPMD kernel on 8 cores; returns (full_output, results)."""
    from concourse import bass_utils
    in_maps = _prep_in_maps(
        inputs["hidden_states"], inputs["position_ids"],
        inputs["Wq"], inputs["Wk"], inputs["Wv"], inputs["Wo"],
    )
    nc = _get_nc()
    res = bass_utils.run_bass_kernel_spmd(
        nc, in_maps, core_ids=list(range(N_CORES)), trace=trace, **kw
    )
    acc = np.zeros((HIDDEN, TOK), dtype=np.float64)
    for r_ in res.results:
        acc += np.asarray(r_["yt"], dtype=np.float64)
    out = np.ascontiguousarray(acc.T.astype(np.float32).reshape(B, S, HIDDEN))
    return out, res


def timing_maps():
    """Small ExternalInputs for the timing-loop variant."""
    import ml_dtypes
    bf16 = ml_dtypes.bfloat16
    cos_t, sin_t = _rope_tables(np.arange(S)[None, :], S)
    return {
        "cos_t": cos_t.astype(bf16), "sin_t": sin_t.astype(bf16),
        "ident": np.eye(128, dtype=np.float32).astype(bf16),
        "ones": np.ones((128, 128), dtype=np.float32),
    }


def kernel(**inputs) -> np.ndarray:
    out, _ = run(inputs, trace=False)
    return out

